# revision 16
# baseline (speedup 1.0000x reference)
"""Trainium2 Bass kernel for NeuralDisCoCirc forward pass.

Problem: L=8 sequential layers; each layer, per sample b:
    z = h @ W[l,b] + bias[l,b];  h = where(mask[l,b], relu(z), z)
Shapes: x [16,1024] f32, weights [8,16,1024,1024] f32,
        biases/masks [8,16,1024].

Strategy (data-parallel over batch, 2 samples per core, 8 cores):
  - Host pre-permutes each core's weight shard to a DMA-friendly layout
    [t=l*2+b, p, ki*1024+j] (p = i%128, ki = i//128) so every per-layer
    weight load is one fully contiguous [128 x 32KB] DMA (4 MB).
  - On-device, h is kept "column-major" ([p, ki], one element per
    partition) so it can serve as the matmul stationary operand.
    Per layer: 16 accumulating matmuls (h chunk [128,1] stationary,
    W chunk [128,512] moving) produce z as a [1,1024] PSUM row;
    ACT copies it to SBUF; 8 K=1 outer-product matmuls transpose it
    back to column-major [128,8]; DVE applies bias + masked relu.
  - Weights stream as float32r (full fp32 bits; PE processes the moving
    operand at 1 cycle/row for N>=256 vs 4 cycles/row for plain fp32).
  - The kernel is memory-bound: 64 MB of weights per core at ~360 GB/s.
"""

import numpy as np

import concourse.bass as bass
import concourse.mybir as mybir
from concourse import bacc
from concourse.tile import TileContext
from concourse.bass_utils import run_bass_kernel_spmd

L = 8          # layers
B = 16         # full batch
D = 1024       # width
NCORES = 8
BC = B // NCORES   # samples per core (2)
NT = L * BC        # (layer, sample) tiles per core (16)
KI = D // 128      # 8 chunks of 128 along the contraction dim
P = 128

F32 = mybir.dt.float32
F32R = mybir.dt.float32r
BF16 = mybir.dt.bfloat16

# "f32r": upload fp32, stream through PE as float32r (fast path)
# "f32" : upload fp32, plain fp32 matmul (4 cycles/row, slower PE)
# "bf16": upload bf16 (half DMA bytes), bf16 matmul
WMODE = "f32r"

_CACHE = {}


def _build(wmode: str) -> bass.Bass:
    wdt = {"bf16": BF16, "f32r": F32R, "f32": F32}[wmode]
    hdt = {"bf16": BF16, "f32r": F32R, "f32": F32}[wmode]

    nc = bacc.Bacc("TRN2", target_bir_lowering=False, debug=False)
    # Declare weight/x DRAM as the matmul dtype directly (f32r has identical
    # bits to f32 on upload) so loads stay on HWDGE with no SWDGE cast.
    w = nc.declare_dram_parameter("w", [NT, P, KI * D], wdt, isOutput=False)
    x = nc.declare_dram_parameter("x", [P, BC * KI], hdt, isOutput=False)
    bm = nc.declare_dram_parameter("bm", [P, NT * 2 * KI], F32, isOutput=False)
    out = nc.declare_dram_parameter("out", [P, BC * KI], F32, isOutput=True)

    with TileContext(nc) as tc:
        with (
            tc.tile_pool(name="wp", bufs=5) as wp,  # per-tag: 5 x 2MB x 2 tags
            tc.tile_pool(name="const", bufs=1) as cp,
            tc.tile_pool(name="hrow", bufs=4) as hrp,
            tc.tile_pool(name="hcol", bufs=4) as hcp,
            tc.tile_pool(name="psr", bufs=2, space="PSUM") as psr,
            tc.tile_pool(name="psc", bufs=2, space="PSUM") as psc,
        ):
            # Weight DMAs are emitted first so the HWDGE rings start
            # streaming W immediately; bm/x go via SWDGE (separate path).
            KH = KI // 2  # ki chunks per half-tile
            LAST = NT - 1
            wtiles = {}
            for t in range(NT):
                if t < LAST:
                    # two 2MB half-tiles, one per HWDGE ring, so
                    # descriptor generation pipelines across both;
                    # alternate ring assignment per tile so slot-release
                    # skew doesn't pile up on one ring
                    wa = wp.tile([P, KH * D], wdt, tag="wa")
                    wb = wp.tile([P, KH * D], wdt, tag="wb")
                    ea, eb = (nc.sync, nc.scalar) if t % 2 == 0 else (
                        nc.scalar, nc.sync)
                    ea.dma_start(out=wa, in_=w[t, :, : KH * D])
                    eb.dma_start(out=wb, in_=w[t, :, KH * D:])
                    wtiles[t] = (wa, wb)
                else:
                    # last tile: 512KB eighths so its matmuls overlap the
                    # tail of the DMA stream (shortens the exposed tail).
                    # Eighths reuse the wa/wb slot tags so pool-slot
                    # recycling keeps them LAST in the HWDGE ring order.
                    qs = []
                    for q in range(8):
                        wq = wp.tile([P, KH * D], wdt,
                                     tag=("wa" if q % 2 == 0 else "wb"))
                        eng = nc.sync if q % 2 == 0 else nc.scalar
                        eng.dma_start(
                            out=wq[:, :D],
                            in_=w[t, :, q * D:(q + 1) * D],
                        )
                        qs.append(wq[:, :D])
                    wtiles[t] = tuple(qs)

            bmt = cp.tile([P, NT * 2 * KI], F32, tag="bm")
            nc.gpsimd.dma_start(out=bmt, in_=bm[:])
            ones = cp.tile([1, 1], F32, tag="ones")
            nc.vector.memset(ones, 1.0)
            outt = cp.tile([P, BC * KI], F32, tag="out")

            xt = cp.tile([P, BC * KI], hdt, tag="x")
            nc.gpsimd.dma_start(out=xt, in_=x[:])
            h = [xt[:, b * KI:(b + 1) * KI] for b in range(BC)]

            for l in range(L):
                for b in range(BC):
                    t = l * BC + b

                    # z row = h @ W : 2 psum groups of 8 accumulating matmuls
                    prow = psr.tile([1, D], F32)
                    cur = h[b]
                    for ki in range(KI):
                        for jb in range(2):
                            wparts = wtiles[t]
                            kper = KI // len(wparts)
                            wh = wparts[ki // kper]
                            ko = ki % kper
                            nc.tensor.matmul(
                                prow[0:1, jb * 512:(jb + 1) * 512],
                                lhsT=cur[:, ki:ki + 1],
                                rhs=wh[:, ko * D + jb * 512: ko * D + jb * 512 + 512],
                                start=(ki == 0),
                                stop=(ki == KI - 1),
                            )

                    # PSUM row -> SBUF row (ACT) chunk-by-chunk, each chunk
                    # immediately transposed to column-major by a K=1
                    # outer-product matmul — pipelines ACT with PE and cuts
                    # the serial tail latency
                    hrow = hrp.tile([1, D], F32)
                    pcol = psc.tile([P, KI], F32)
                    for ki in range(KI):
                        nc.vector.tensor_copy(
                            out=hrow[0:1, ki * P:(ki + 1) * P],
                            in_=prow[0:1, ki * P:(ki + 1) * P],
                        )
                        nc.tensor.matmul(
                            pcol[:, ki:ki + 1],
                            lhsT=hrow[0:1, ki * P:(ki + 1) * P],
                            rhs=ones[0:1, 0:1],
                            start=True,
                            stop=True,
                        )

                    # bias + masked relu:  zb = z + bias;
                    # h = zb - mask * min(zb, 0)
                    bias_ap = bmt[:, t * 2 * KI: t * 2 * KI + KI]
                    mask_ap = bmt[:, t * 2 * KI + KI: (t + 1) * 2 * KI]
                    zb = hcp.tile([P, KI], F32, tag="zb")
                    nc.vector.tensor_add(out=zb, in0=pcol[:], in1=bias_ap)
                    tmp = hcp.tile([P, KI], F32, tag="tmp")
                    nc.vector.scalar_tensor_tensor(
                        out=tmp,
                        in0=zb,
                        scalar=0.0,
                        in1=mask_ap,
                        op0=mybir.AluOpType.min,
                        op1=mybir.AluOpType.mult,
                    )
                    if l == L - 1:
                        nc.vector.tensor_sub(
                            out=outt[:, b * KI:(b + 1) * KI], in0=zb, in1=tmp
                        )
                        # ship each sample's output as soon as it's done.
                        # b=0 goes via SWDGE (HWDGE rings still stream
                        # weights then); the final b=1 write uses the sync
                        # ring, which is idle by that point and has lower
                        # fixed latency than SWDGE.
                        eng = nc.gpsimd if b == 0 else nc.sync
                        eng.dma_start(
                            out=out[:, b * KI:(b + 1) * KI],
                            in_=outt[:, b * KI:(b + 1) * KI],
                        )
                    else:
                        hnew = hcp.tile([P, KI], hdt, tag="h")
                        nc.vector.tensor_sub(out=hnew, in0=zb, in1=tmp)
                        h[b] = hnew
    nc.finalize()
    return nc


def _get_nc():
    if WMODE not in _CACHE:
        _CACHE[WMODE] = _build(WMODE)
    return _CACHE[WMODE]


def _prep_core_inputs(c, x, weights, biases, masks):
    b0 = c * BC
    # weights[l, b, i, j], i = ki*128 + p  ->  [t, p, ki*1024 + j]
    wc = weights[:, b0:b0 + BC].reshape(L, BC, KI, P, D)
    wc = np.ascontiguousarray(wc.transpose(0, 1, 3, 2, 4)).reshape(NT, P, KI * D)
    if WMODE == "bf16":
        import ml_dtypes
        wc = wc.astype(ml_dtypes.bfloat16)
    # x[b, ki*128+p] -> [p, b*KI + ki]
    xc = x[b0:b0 + BC].reshape(BC, KI, P)
    xc = np.ascontiguousarray(xc.transpose(2, 0, 1)).reshape(P, BC * KI)
    # bias/mask [l, b, ki*128+p] -> [p, (t, {bias,mask}, ki)]
    bc = biases[:, b0:b0 + BC].reshape(L, BC, KI, P).transpose(3, 0, 1, 2)
    mc = masks[:, b0:b0 + BC].astype(np.float32).reshape(L, BC, KI, P)
    mc = mc.transpose(3, 0, 1, 2)
    # stack along a new axis after (l, b): [p, l, b, 2, ki]
    bmc = np.stack([bc, mc], axis=3)  # [p, L, BC, 2, KI]
    bmc = np.ascontiguousarray(bmc).reshape(P, NT * 2 * KI)
    return {"w": wc, "x": xc, "bm": bmc}


def _run(inputs: dict, trace: bool = False, trace_cores=None):
    x = np.asarray(inputs["x"], dtype=np.float32)
    weights = np.asarray(inputs["weights"], dtype=np.float32)
    biases = np.asarray(inputs["biases"], dtype=np.float32)
    masks = np.asarray(inputs["masks"])

    nc = _get_nc()
    in_maps = [
        _prep_core_inputs(c, x, weights, biases, masks) for c in range(NCORES)
    ]
    kw = {}
    if trace_cores is not None:
        kw["trace_cores"] = trace_cores
    res = run_bass_kernel_spmd(
        nc, in_maps, core_ids=list(range(NCORES)), trace=trace, **kw
    )
    outs = []
    for c in range(NCORES):
        oc = res.results[c]["out"]  # [P, BC*KI]
        oc = oc.reshape(P, BC, KI).transpose(1, 2, 0).reshape(BC, D)
        outs.append(oc)
    full = np.concatenate(outs, axis=0).astype(np.float32)
    return full, res


def kernel(**inputs) -> np.ndarray:
    full, _ = _run(inputs, trace=False)
    return full


# revision 17
# speedup vs baseline: 1.0242x; 1.0242x over previous
"""Trainium2 Bass kernel for NeuralDisCoCirc forward pass.

Problem: L=8 sequential layers; each layer, per sample b:
    z = h @ W[l,b] + bias[l,b];  h = where(mask[l,b], relu(z), z)
Shapes: x [16,1024] f32, weights [8,16,1024,1024] f32,
        biases/masks [8,16,1024].

Strategy (data-parallel over batch, 2 samples per core, 8 cores):
  - Host pre-permutes each core's weight shard to a DMA-friendly layout
    [t=l*2+b, p, ki*1024+j] (p = i%128, ki = i//128) so every per-layer
    weight load is one fully contiguous [128 x 32KB] DMA (4 MB).
  - On-device, h is kept "column-major" ([p, ki], one element per
    partition) so it can serve as the matmul stationary operand.
    Per layer: 16 accumulating matmuls (h chunk [128,1] stationary,
    W chunk [128,512] moving) produce z as a [1,1024] PSUM row;
    ACT copies it to SBUF; 8 K=1 outer-product matmuls transpose it
    back to column-major [128,8]; DVE applies bias + masked relu.
  - Weights stream as float32r (full fp32 bits; PE processes the moving
    operand at 1 cycle/row for N>=256 vs 4 cycles/row for plain fp32).
  - The kernel is memory-bound: 64 MB of weights per core at ~360 GB/s.
"""

import numpy as np

import concourse.bass as bass
import concourse.mybir as mybir
from concourse import bacc
from concourse.tile import TileContext
from concourse.bass_utils import run_bass_kernel_spmd

L = 8          # layers
B = 16         # full batch
D = 1024       # width
NCORES = 8
BC = B // NCORES   # samples per core (2)
NT = L * BC        # (layer, sample) tiles per core (16)
KI = D // 128      # 8 chunks of 128 along the contraction dim
P = 128

F32 = mybir.dt.float32
F32R = mybir.dt.float32r
BF16 = mybir.dt.bfloat16

# "f32r": upload fp32, stream through PE as float32r (fast path)
# "f32" : upload fp32, plain fp32 matmul (4 cycles/row, slower PE)
# "bf16": upload bf16 (half DMA bytes), bf16 matmul
WMODE = "f32r"

_CACHE = {}


def _build(wmode: str) -> bass.Bass:
    wdt = {"bf16": BF16, "f32r": F32R, "f32": F32}[wmode]
    hdt = {"bf16": BF16, "f32r": F32R, "f32": F32}[wmode]

    nc = bacc.Bacc("TRN2", target_bir_lowering=False, debug=False)
    # Declare weight/x DRAM as the matmul dtype directly (f32r has identical
    # bits to f32 on upload) so loads stay on HWDGE with no SWDGE cast.
    w = nc.declare_dram_parameter("w", [NT, P, KI * D], wdt, isOutput=False)
    x = nc.declare_dram_parameter("x", [P, BC * KI], hdt, isOutput=False)
    bm = nc.declare_dram_parameter("bm", [P, NT * 2 * KI], F32, isOutput=False)
    out = nc.declare_dram_parameter("out", [P, BC * KI], F32, isOutput=True)

    with TileContext(nc) as tc:
        with (
            tc.tile_pool(name="wp", bufs=4) as wp,  # per-tag: 4 x 2MB x 2 tags
            tc.tile_pool(name="const", bufs=1) as cp,
            tc.tile_pool(name="hrow", bufs=4) as hrp,
            tc.tile_pool(name="hcol", bufs=4) as hcp,
            tc.tile_pool(name="psr", bufs=2, space="PSUM") as psr,
            tc.tile_pool(name="psc", bufs=2, space="PSUM") as psc,
        ):
            # Weight DMAs are emitted first so the HWDGE rings start
            # streaming W immediately; bm/x go via SWDGE (separate path).
            KH = KI // 2  # ki chunks per half-tile
            LAST = NT - 1
            wtiles = {}
            for t in range(NT):
                if t < LAST:
                    # two 2MB half-tiles, one per HWDGE ring, so
                    # descriptor generation pipelines across both;
                    # alternate ring assignment per tile so slot-release
                    # skew doesn't pile up on one ring
                    wa = wp.tile([P, KH * D], wdt, tag="wa")
                    wb = wp.tile([P, KH * D], wdt, tag="wb")
                    ea, eb = (nc.sync, nc.scalar) if t % 2 == 0 else (
                        nc.scalar, nc.sync)
                    ea.dma_start(out=wa, in_=w[t, :, : KH * D])
                    eb.dma_start(out=wb, in_=w[t, :, KH * D:])
                    wtiles[t] = (wa, wb)
                else:
                    # last tile: 512KB eighths so its matmuls overlap the
                    # tail of the DMA stream (shortens the exposed tail).
                    # Eighths reuse the wa/wb slot tags so pool-slot
                    # recycling keeps them LAST in the HWDGE ring order.
                    qs = []
                    for q in range(8):
                        wq = wp.tile([P, KH * D], wdt,
                                     tag=("wa" if q % 2 == 0 else "wb"))
                        eng = nc.sync if q % 2 == 0 else nc.scalar
                        eng.dma_start(
                            out=wq[:, :D],
                            in_=w[t, :, q * D:(q + 1) * D],
                        )
                        qs.append(wq[:, :D])
                    wtiles[t] = tuple(qs)

            bmt = cp.tile([P, NT * 2 * KI], F32, tag="bm")
            nc.gpsimd.dma_start(out=bmt, in_=bm[:])
            ones = cp.tile([1, 1], F32, tag="ones")
            nc.vector.memset(ones, 1.0)
            outt = cp.tile([P, BC * KI], F32, tag="out")

            xt = cp.tile([P, BC * KI], hdt, tag="x")
            nc.gpsimd.dma_start(out=xt, in_=x[:])
            h = [xt[:, b * KI:(b + 1) * KI] for b in range(BC)]

            for l in range(L):
                for b in range(BC):
                    t = l * BC + b

                    # z row = h @ W : 2 psum groups of 8 accumulating matmuls
                    prow = psr.tile([1, D], F32)
                    cur = h[b]
                    for ki in range(KI):
                        for jb in range(2):
                            wparts = wtiles[t]
                            kper = KI // len(wparts)
                            wh = wparts[ki // kper]
                            ko = ki % kper
                            nc.tensor.matmul(
                                prow[0:1, jb * 512:(jb + 1) * 512],
                                lhsT=cur[:, ki:ki + 1],
                                rhs=wh[:, ko * D + jb * 512: ko * D + jb * 512 + 512],
                                start=(ki == 0),
                                stop=(ki == KI - 1),
                            )

                    # PSUM row -> SBUF row (ACT) chunk-by-chunk, each chunk
                    # immediately transposed to column-major by a K=1
                    # outer-product matmul — pipelines ACT with PE and cuts
                    # the serial tail latency
                    hrow = hrp.tile([1, D], F32)
                    pcol = psc.tile([P, KI], F32)
                    for ki in range(KI):
                        nc.vector.tensor_copy(
                            out=hrow[0:1, ki * P:(ki + 1) * P],
                            in_=prow[0:1, ki * P:(ki + 1) * P],
                        )
                        nc.tensor.matmul(
                            pcol[:, ki:ki + 1],
                            lhsT=hrow[0:1, ki * P:(ki + 1) * P],
                            rhs=ones[0:1, 0:1],
                            start=True,
                            stop=True,
                        )

                    # bias + masked relu:  zb = z + bias;
                    # h = zb - mask * min(zb, 0)
                    bias_ap = bmt[:, t * 2 * KI: t * 2 * KI + KI]
                    mask_ap = bmt[:, t * 2 * KI + KI: (t + 1) * 2 * KI]
                    zb = hcp.tile([P, KI], F32, tag="zb")
                    nc.vector.tensor_add(out=zb, in0=pcol[:], in1=bias_ap)
                    tmp = hcp.tile([P, KI], F32, tag="tmp")
                    nc.vector.scalar_tensor_tensor(
                        out=tmp,
                        in0=zb,
                        scalar=0.0,
                        in1=mask_ap,
                        op0=mybir.AluOpType.min,
                        op1=mybir.AluOpType.mult,
                    )
                    if l == L - 1:
                        nc.vector.tensor_sub(
                            out=outt[:, b * KI:(b + 1) * KI], in0=zb, in1=tmp
                        )
                        # ship each sample's output as soon as it's done.
                        # b=0 goes via SWDGE (HWDGE rings still stream
                        # weights then); the final b=1 write uses the sync
                        # ring, which is idle by that point and has lower
                        # fixed latency than SWDGE.
                        eng = nc.gpsimd if b == 0 else nc.sync
                        eng.dma_start(
                            out=out[:, b * KI:(b + 1) * KI],
                            in_=outt[:, b * KI:(b + 1) * KI],
                        )
                    else:
                        hnew = hcp.tile([P, KI], hdt, tag="h")
                        nc.vector.tensor_sub(out=hnew, in0=zb, in1=tmp)
                        h[b] = hnew
    nc.finalize()
    return nc


def _get_nc():
    if WMODE not in _CACHE:
        _CACHE[WMODE] = _build(WMODE)
    return _CACHE[WMODE]


def _prep_core_inputs(c, x, weights, biases, masks):
    b0 = c * BC
    # weights[l, b, i, j], i = ki*128 + p  ->  [t, p, ki*1024 + j]
    wc = weights[:, b0:b0 + BC].reshape(L, BC, KI, P, D)
    wc = np.ascontiguousarray(wc.transpose(0, 1, 3, 2, 4)).reshape(NT, P, KI * D)
    if WMODE == "bf16":
        import ml_dtypes
        wc = wc.astype(ml_dtypes.bfloat16)
    # x[b, ki*128+p] -> [p, b*KI + ki]
    xc = x[b0:b0 + BC].reshape(BC, KI, P)
    xc = np.ascontiguousarray(xc.transpose(2, 0, 1)).reshape(P, BC * KI)
    # bias/mask [l, b, ki*128+p] -> [p, (t, {bias,mask}, ki)]
    bc = biases[:, b0:b0 + BC].reshape(L, BC, KI, P).transpose(3, 0, 1, 2)
    mc = masks[:, b0:b0 + BC].astype(np.float32).reshape(L, BC, KI, P)
    mc = mc.transpose(3, 0, 1, 2)
    # stack along a new axis after (l, b): [p, l, b, 2, ki]
    bmc = np.stack([bc, mc], axis=3)  # [p, L, BC, 2, KI]
    bmc = np.ascontiguousarray(bmc).reshape(P, NT * 2 * KI)
    return {"w": wc, "x": xc, "bm": bmc}


def _run(inputs: dict, trace: bool = False, trace_cores=None):
    x = np.asarray(inputs["x"], dtype=np.float32)
    weights = np.asarray(inputs["weights"], dtype=np.float32)
    biases = np.asarray(inputs["biases"], dtype=np.float32)
    masks = np.asarray(inputs["masks"])

    nc = _get_nc()
    in_maps = [
        _prep_core_inputs(c, x, weights, biases, masks) for c in range(NCORES)
    ]
    kw = {}
    if trace_cores is not None:
        kw["trace_cores"] = trace_cores
    res = run_bass_kernel_spmd(
        nc, in_maps, core_ids=list(range(NCORES)), trace=trace, **kw
    )
    outs = []
    for c in range(NCORES):
        oc = res.results[c]["out"]  # [P, BC*KI]
        oc = oc.reshape(P, BC, KI).transpose(1, 2, 0).reshape(BC, D)
        outs.append(oc)
    full = np.concatenate(outs, axis=0).astype(np.float32)
    return full, res


def kernel(**inputs) -> np.ndarray:
    full, _ = _run(inputs, trace=False)
    return full


# revision 18
# speedup vs baseline: 1.0544x; 1.0295x over previous
"""Trainium2 Bass kernel for NeuralDisCoCirc forward pass.

Problem: L=8 sequential layers; each layer, per sample b:
    z = h @ W[l,b] + bias[l,b];  h = where(mask[l,b], relu(z), z)
Shapes: x [16,1024] f32, weights [8,16,1024,1024] f32,
        biases/masks [8,16,1024].

Strategy (data-parallel over batch, 2 samples per core, 8 cores):
  - Host pre-permutes each core's weight shard to a DMA-friendly layout
    [t=l*2+b, p, ki*1024+j] (p = i%128, ki = i//128) so per-layer weight
    loads are fully contiguous [128 x 8KB] DMAs (2 MB halves), streamed
    on both HWDGE rings (sync + scalar) with 4-deep prefetch; the last
    tile is split into 512KB eighths so its matmuls overlap the DMA tail.
  - On-device, h is kept "column-major" ([p, ki], one element per
    partition) so it can serve as the matmul stationary operand.
    Per layer: 16 accumulating matmuls (h chunk [128,1] stationary,
    W chunk [128,512] moving) produce z as a [1,1024] PSUM row;
    DVE copies it to SBUF chunk-by-chunk, each chunk transposed back to
    column-major [128,8] by a K=1 outer-product matmul (pipelined);
    DVE applies bias + masked relu: h = zb - mask*min(zb, 0).
  - Weights stream as float32r (same bits as fp32, TF32-like rounding in
    the PE, ~1.5e-4 rel err per matmul; PE processes the moving operand
    at 1 cycle/row for N>=256 vs 4 cycles/row for plain fp32).
  - The kernel is memory-bound: 64 MB of weights per core; both cores of
    an HBM pair stream concurrently, so the roofline is ~128MB/716GB/s
    ~= 188 us per pair. Measured ~183-210 us per core end to end.
"""

import numpy as np

import concourse.bass as bass
import concourse.mybir as mybir
from concourse import bacc
from concourse.tile import TileContext
from concourse.bass_utils import run_bass_kernel_spmd

L = 8          # layers
B = 16         # full batch
D = 1024       # width
NCORES = 8
BC = B // NCORES   # samples per core (2)
NT = L * BC        # (layer, sample) tiles per core (16)
KI = D // 128      # 8 chunks of 128 along the contraction dim
P = 128

F32 = mybir.dt.float32
F32R = mybir.dt.float32r
BF16 = mybir.dt.bfloat16

# "f32r": upload fp32, stream through PE as float32r (fast path)
# "f32" : upload fp32, plain fp32 matmul (4 cycles/row, slower PE)
# "bf16": upload bf16 (half DMA bytes), bf16 matmul
WMODE = "f32r"

_CACHE = {}


def _build(wmode: str) -> bass.Bass:
    wdt = {"bf16": BF16, "f32r": F32R, "f32": F32}[wmode]
    hdt = {"bf16": BF16, "f32r": F32R, "f32": F32}[wmode]

    nc = bacc.Bacc("TRN2", target_bir_lowering=False, debug=False)
    # Declare weight/x DRAM as the matmul dtype directly (f32r has identical
    # bits to f32 on upload) so loads stay on HWDGE with no SWDGE cast.
    w = nc.declare_dram_parameter("w", [NT, P, KI * D], wdt, isOutput=False)
    x = nc.declare_dram_parameter("x", [P, BC * KI], hdt, isOutput=False)
    bm = nc.declare_dram_parameter("bm", [P, NT * 2 * KI], F32, isOutput=False)
    out = nc.declare_dram_parameter("out", [P, BC * KI], F32, isOutput=True)

    with TileContext(nc) as tc:
        with (
            tc.tile_pool(name="wp", bufs=4) as wp,  # per-tag: 4 x 2MB x 2 tags
            tc.tile_pool(name="const", bufs=1) as cp,
            tc.tile_pool(name="hrow", bufs=4) as hrp,
            tc.tile_pool(name="hcol", bufs=4) as hcp,
            tc.tile_pool(name="psr", bufs=2, space="PSUM") as psr,
            tc.tile_pool(name="psc", bufs=2, space="PSUM") as psc,
        ):
            # Weight DMAs are emitted first so the HWDGE rings start
            # streaming W immediately; bm/x go via SWDGE (separate path).
            KH = KI // 2  # ki chunks per half-tile
            LAST = NT - 1
            wtiles = {}
            for t in range(NT):
                if t < LAST:
                    # two 2MB half-tiles, one per HWDGE ring, so
                    # descriptor generation pipelines across both;
                    # alternate ring assignment per tile so slot-release
                    # skew doesn't pile up on one ring
                    wa = wp.tile([P, KH * D], wdt, tag="wa")
                    wb = wp.tile([P, KH * D], wdt, tag="wb")
                    ea, eb = (nc.sync, nc.scalar) if t % 2 == 0 else (
                        nc.scalar, nc.sync)
                    ea.dma_start(out=wa, in_=w[t, :, : KH * D])
                    eb.dma_start(out=wb, in_=w[t, :, KH * D:])
                    wtiles[t] = (wa, wb)
                else:
                    # last tile: 512KB eighths so its matmuls overlap the
                    # tail of the DMA stream (shortens the exposed tail).
                    # Eighths reuse the wa/wb slot tags so pool-slot
                    # recycling keeps them LAST in the HWDGE ring order.
                    qs = []
                    for q in range(8):
                        wq = wp.tile([P, KH * D], wdt,
                                     tag=("wa" if q % 2 == 0 else "wb"))
                        eng = nc.sync if q % 2 == 0 else nc.scalar
                        eng.dma_start(
                            out=wq[:, :D],
                            in_=w[t, :, q * D:(q + 1) * D],
                        )
                        qs.append(wq[:, :D])
                    wtiles[t] = tuple(qs)

            bmt = cp.tile([P, NT * 2 * KI], F32, tag="bm")
            nc.gpsimd.dma_start(out=bmt, in_=bm[:])
            ones = cp.tile([1, 1], F32, tag="ones")
            nc.vector.memset(ones, 1.0)
            outt = cp.tile([P, BC * KI], F32, tag="out")

            xt = cp.tile([P, BC * KI], hdt, tag="x")
            nc.gpsimd.dma_start(out=xt, in_=x[:])
            h = [xt[:, b * KI:(b + 1) * KI] for b in range(BC)]

            for l in range(L):
                for b in range(BC):
                    t = l * BC + b

                    # z row = h @ W : 2 psum groups of 8 accumulating matmuls
                    prow = psr.tile([1, D], F32)
                    cur = h[b]
                    for ki in range(KI):
                        for jb in range(2):
                            wparts = wtiles[t]
                            kper = KI // len(wparts)
                            wh = wparts[ki // kper]
                            ko = ki % kper
                            nc.tensor.matmul(
                                prow[0:1, jb * 512:(jb + 1) * 512],
                                lhsT=cur[:, ki:ki + 1],
                                rhs=wh[:, ko * D + jb * 512: ko * D + jb * 512 + 512],
                                start=(ki == 0),
                                stop=(ki == KI - 1),
                            )

                    # PSUM row -> SBUF row (ACT) chunk-by-chunk, each chunk
                    # immediately transposed to column-major by a K=1
                    # outer-product matmul — pipelines ACT with PE and cuts
                    # the serial tail latency
                    hrow = hrp.tile([1, D], F32)
                    pcol = psc.tile([P, KI], F32)
                    for ki in range(KI):
                        nc.vector.tensor_copy(
                            out=hrow[0:1, ki * P:(ki + 1) * P],
                            in_=prow[0:1, ki * P:(ki + 1) * P],
                        )
                        nc.tensor.matmul(
                            pcol[:, ki:ki + 1],
                            lhsT=hrow[0:1, ki * P:(ki + 1) * P],
                            rhs=ones[0:1, 0:1],
                            start=True,
                            stop=True,
                        )

                    # bias + masked relu:  zb = z + bias;
                    # h = zb - mask * min(zb, 0)
                    bias_ap = bmt[:, t * 2 * KI: t * 2 * KI + KI]
                    mask_ap = bmt[:, t * 2 * KI + KI: (t + 1) * 2 * KI]
                    zb = hcp.tile([P, KI], F32, tag="zb")
                    nc.vector.tensor_add(out=zb, in0=pcol[:], in1=bias_ap)
                    tmp = hcp.tile([P, KI], F32, tag="tmp")
                    nc.vector.scalar_tensor_tensor(
                        out=tmp,
                        in0=zb,
                        scalar=0.0,
                        in1=mask_ap,
                        op0=mybir.AluOpType.min,
                        op1=mybir.AluOpType.mult,
                    )
                    if l == L - 1:
                        nc.vector.tensor_sub(
                            out=outt[:, b * KI:(b + 1) * KI], in0=zb, in1=tmp
                        )
                        # ship each sample's output as soon as it's done.
                        # b=0 goes via SWDGE (HWDGE rings still stream
                        # weights then); the final b=1 write uses the sync
                        # ring, which is idle by that point and has lower
                        # fixed latency than SWDGE.
                        eng = nc.gpsimd if b == 0 else nc.sync
                        eng.dma_start(
                            out=out[:, b * KI:(b + 1) * KI],
                            in_=outt[:, b * KI:(b + 1) * KI],
                        )
                    else:
                        hnew = hcp.tile([P, KI], hdt, tag="h")
                        nc.vector.tensor_sub(out=hnew, in0=zb, in1=tmp)
                        h[b] = hnew
    nc.finalize()
    return nc


def _get_nc():
    if WMODE not in _CACHE:
        _CACHE[WMODE] = _build(WMODE)
    return _CACHE[WMODE]


def _prep_core_inputs(c, x, weights, biases, masks):
    b0 = c * BC
    # weights[l, b, i, j], i = ki*128 + p  ->  [t, p, ki*1024 + j]
    wc = weights[:, b0:b0 + BC].reshape(L, BC, KI, P, D)
    wc = np.ascontiguousarray(wc.transpose(0, 1, 3, 2, 4)).reshape(NT, P, KI * D)
    if WMODE == "bf16":
        import ml_dtypes
        wc = wc.astype(ml_dtypes.bfloat16)
    # x[b, ki*128+p] -> [p, b*KI + ki]
    xc = x[b0:b0 + BC].reshape(BC, KI, P)
    xc = np.ascontiguousarray(xc.transpose(2, 0, 1)).reshape(P, BC * KI)
    # bias/mask [l, b, ki*128+p] -> [p, (t, {bias,mask}, ki)]
    bc = biases[:, b0:b0 + BC].reshape(L, BC, KI, P).transpose(3, 0, 1, 2)
    mc = masks[:, b0:b0 + BC].astype(np.float32).reshape(L, BC, KI, P)
    mc = mc.transpose(3, 0, 1, 2)
    # stack along a new axis after (l, b): [p, l, b, 2, ki]
    bmc = np.stack([bc, mc], axis=3)  # [p, L, BC, 2, KI]
    bmc = np.ascontiguousarray(bmc).reshape(P, NT * 2 * KI)
    return {"w": wc, "x": xc, "bm": bmc}


def _run(inputs: dict, trace: bool = False, trace_cores=None):
    x = np.asarray(inputs["x"], dtype=np.float32)
    weights = np.asarray(inputs["weights"], dtype=np.float32)
    biases = np.asarray(inputs["biases"], dtype=np.float32)
    masks = np.asarray(inputs["masks"])

    nc = _get_nc()
    in_maps = [
        _prep_core_inputs(c, x, weights, biases, masks) for c in range(NCORES)
    ]
    kw = {}
    if trace_cores is not None:
        kw["trace_cores"] = trace_cores
    res = run_bass_kernel_spmd(
        nc, in_maps, core_ids=list(range(NCORES)), trace=trace, **kw
    )
    outs = []
    for c in range(NCORES):
        oc = res.results[c]["out"]  # [P, BC*KI]
        oc = oc.reshape(P, BC, KI).transpose(1, 2, 0).reshape(BC, D)
        outs.append(oc)
    full = np.concatenate(outs, axis=0).astype(np.float32)
    return full, res


def kernel(**inputs) -> np.ndarray:
    full, _ = _run(inputs, trace=False)
    return full


# revision 19
# speedup vs baseline: 1.1677x; 1.1075x over previous
"""Trainium2 Bass kernel for NeuralDisCoCirc forward pass.

Problem: L=8 sequential layers; each layer, per sample b:
    z = h @ W[l,b] + bias[l,b];  h = where(mask[l,b], relu(z), z)
Shapes: x [16,1024] f32, weights [8,16,1024,1024] f32,
        biases/masks [8,16,1024].

Strategy (data-parallel over batch, 2 samples per core, 8 cores):
  - Host pre-permutes each core's weight shard to a DMA-friendly layout
    [t=l*2+b, p, ki*1024+j] (p = i%128, ki = i//128) so per-layer weight
    loads are fully contiguous [128 x 8KB] DMAs (2 MB halves), streamed
    on both HWDGE rings (sync + scalar) with 4-deep prefetch; the last
    tile is split into 512KB eighths so its matmuls overlap the DMA tail.
  - On-device, h is kept "column-major" ([p, ki], one element per
    partition) so it can serve as the matmul stationary operand.
    Per layer: 16 accumulating matmuls (h chunk [128,1] stationary,
    W chunk [128,512] moving) produce z as a [1,1024] PSUM row;
    DVE copies it to SBUF chunk-by-chunk, each chunk transposed back to
    column-major [128,8] by a K=1 outer-product matmul (pipelined);
    DVE applies bias + masked relu: h = zb - mask*min(zb, 0).
  - Weights stream as float32r (same bits as fp32, TF32-like rounding in
    the PE, ~1.5e-4 rel err per matmul; PE processes the moving operand
    at 1 cycle/row for N>=256 vs 4 cycles/row for plain fp32).
  - The kernel is memory-bound: 64 MB of weights per core; both cores of
    an HBM pair stream concurrently, so the roofline is ~128MB/716GB/s
    ~= 188 us per pair. Measured ~183-210 us per core end to end.
"""

import numpy as np

import concourse.bass as bass
import concourse.mybir as mybir
from concourse import bacc
from concourse.tile import TileContext
from concourse.bass_utils import run_bass_kernel_spmd

L = 8          # layers
B = 16         # full batch
D = 1024       # width
NCORES = 8
BC = B // NCORES   # samples per core (2)
NT = L * BC        # (layer, sample) tiles per core (16)
KI = D // 128      # 8 chunks of 128 along the contraction dim
P = 128

F32 = mybir.dt.float32
F32R = mybir.dt.float32r
BF16 = mybir.dt.bfloat16

# "f32r": upload fp32, stream through PE as float32r (fast path)
# "f32" : upload fp32, plain fp32 matmul (4 cycles/row, slower PE)
# "bf16": upload bf16 (half DMA bytes), bf16 matmul
WMODE = "f32r"

_CACHE = {}


def _build(wmode: str) -> bass.Bass:
    wdt = {"bf16": BF16, "f32r": F32R, "f32": F32}[wmode]
    hdt = {"bf16": BF16, "f32r": F32R, "f32": F32}[wmode]

    nc = bacc.Bacc("TRN2", target_bir_lowering=False, debug=False)
    # Declare weight/x DRAM as the matmul dtype directly (f32r has identical
    # bits to f32 on upload) so loads stay on HWDGE with no SWDGE cast.
    w = nc.declare_dram_parameter("w", [NT, P, KI * D], wdt, isOutput=False)
    x = nc.declare_dram_parameter("x", [P, BC * KI], hdt, isOutput=False)
    bm = nc.declare_dram_parameter("bm", [P, NT * 2 * KI], F32, isOutput=False)
    out = nc.declare_dram_parameter("out", [P, BC * KI], F32, isOutput=True)

    with TileContext(nc) as tc:
        with (
            tc.tile_pool(name="wp", bufs=4) as wp,  # per-tag: 4 x 2MB x 2 tags
            tc.tile_pool(name="const", bufs=1) as cp,
            tc.tile_pool(name="hrow", bufs=4) as hrp,
            tc.tile_pool(name="hcol", bufs=4) as hcp,
            tc.tile_pool(name="psr", bufs=2, space="PSUM") as psr,
            tc.tile_pool(name="psc", bufs=2, space="PSUM") as psc,
        ):
            # Weight DMAs are emitted first so the HWDGE rings start
            # streaming W immediately; bm/x go via SWDGE (separate path).
            KH = KI // 2  # ki chunks per half-tile
            LAST = NT - 1
            wtiles = {}
            for t in range(NT):
                if t < LAST:
                    # two 2MB half-tiles, one per HWDGE ring, so
                    # descriptor generation pipelines across both;
                    # alternate ring assignment per tile so slot-release
                    # skew doesn't pile up on one ring
                    wa = wp.tile([P, KH * D], wdt, tag="wa")
                    wb = wp.tile([P, KH * D], wdt, tag="wb")
                    ea, eb = (nc.sync, nc.scalar) if t % 2 == 0 else (
                        nc.scalar, nc.sync)
                    ea.dma_start(out=wa, in_=w[t, :, : KH * D])
                    eb.dma_start(out=wb, in_=w[t, :, KH * D:])
                    wtiles[t] = (wa, wb)
                else:
                    # last tile: host re-laid it out jb-major
                    # ([p, jb*4096 + ki*512 + j']), streamed as 8
                    # contiguous 512KB blocks, jb0's four blocks first.
                    # So the jb0 PSUM group closes at the tile's halfway
                    # point and its transpose chain overlaps the jb1
                    # stream — only jb1's half-chain is exposed at the
                    # very end.  Blocks reuse the wa/wb slot tags so
                    # pool-slot recycling keeps them LAST in ring order.
                    qs = []
                    for q in range(8):
                        wq = wp.tile([P, KH * D], wdt,
                                     tag=("wa" if q % 2 == 0 else "wb"))
                        eng = nc.sync if q % 2 == 0 else nc.scalar
                        eng.dma_start(
                            out=wq[:, :D],
                            in_=w[t, :, q * D:(q + 1) * D],
                        )
                        qs.append(wq[:, :D])
                    wtiles[t] = tuple(qs)

            bmt = cp.tile([P, NT * 2 * KI], F32, tag="bm")
            nc.gpsimd.dma_start(out=bmt, in_=bm[:])
            ones = cp.tile([1, 1], F32, tag="ones")
            nc.vector.memset(ones, 1.0)
            outt = cp.tile([P, BC * KI], F32, tag="out")

            xt = cp.tile([P, BC * KI], hdt, tag="x")
            nc.gpsimd.dma_start(out=xt, in_=x[:])
            h = [xt[:, b * KI:(b + 1) * KI] for b in range(BC)]

            for l in range(L):
                for b in range(BC):
                    t = l * BC + b

                    # z row = h @ W : 2 psum groups of 8 accumulating
                    # matmuls.  jb-major order: the jb0 group (PSUM bank 0)
                    # closes after 8 MMs, so its transpose chain can start
                    # while the jb1 MMs are still running.
                    prow = psr.tile([1, D], F32)
                    cur = h[b]
                    for jb in range(2):
                        for ki in range(KI):
                            if t < LAST:
                                wh = wtiles[t][0] if ki < KH else wtiles[t][1]
                                rhs = wh[:, (ki % KH) * D + jb * 512:
                                          (ki % KH) * D + jb * 512 + 512]
                            else:
                                blk = wtiles[t][jb * 4 + ki // 2]
                                rhs = blk[:, (ki % 2) * 512:
                                          (ki % 2) * 512 + 512]
                            nc.tensor.matmul(
                                prow[0:1, jb * 512:(jb + 1) * 512],
                                lhsT=cur[:, ki:ki + 1],
                                rhs=rhs,
                                start=(ki == 0),
                                stop=(ki == KI - 1),
                            )

                    # PSUM row -> SBUF row (ACT) chunk-by-chunk, each chunk
                    # immediately transposed to column-major by a K=1
                    # outer-product matmul — pipelines ACT with PE and cuts
                    # the serial tail latency
                    hrow = hrp.tile([1, D], F32)
                    pcol = psc.tile([P, KI], F32)
                    for ki in range(KI):
                        nc.vector.tensor_copy(
                            out=hrow[0:1, ki * P:(ki + 1) * P],
                            in_=prow[0:1, ki * P:(ki + 1) * P],
                        )
                        nc.tensor.matmul(
                            pcol[:, ki:ki + 1],
                            lhsT=hrow[0:1, ki * P:(ki + 1) * P],
                            rhs=ones[0:1, 0:1],
                            start=True,
                            stop=True,
                        )

                    # bias + masked relu:  zb = z + bias;
                    # h = zb - mask * min(zb, 0)
                    bias_ap = bmt[:, t * 2 * KI: t * 2 * KI + KI]
                    mask_ap = bmt[:, t * 2 * KI + KI: (t + 1) * 2 * KI]
                    zb = hcp.tile([P, KI], F32, tag="zb")
                    nc.vector.tensor_add(out=zb, in0=pcol[:], in1=bias_ap)
                    tmp = hcp.tile([P, KI], F32, tag="tmp")
                    nc.vector.scalar_tensor_tensor(
                        out=tmp,
                        in0=zb,
                        scalar=0.0,
                        in1=mask_ap,
                        op0=mybir.AluOpType.min,
                        op1=mybir.AluOpType.mult,
                    )
                    if l == L - 1:
                        nc.vector.tensor_sub(
                            out=outt[:, b * KI:(b + 1) * KI], in0=zb, in1=tmp
                        )
                        # ship each sample's output as soon as it's done.
                        # b=0 goes via SWDGE (HWDGE rings still stream
                        # weights then); the final b=1 write uses the sync
                        # ring, which is idle by that point and has lower
                        # fixed latency than SWDGE.
                        eng = nc.gpsimd if b == 0 else nc.sync
                        eng.dma_start(
                            out=out[:, b * KI:(b + 1) * KI],
                            in_=outt[:, b * KI:(b + 1) * KI],
                        )
                    else:
                        hnew = hcp.tile([P, KI], hdt, tag="h")
                        nc.vector.tensor_sub(out=hnew, in0=zb, in1=tmp)
                        h[b] = hnew
    nc.finalize()
    return nc


def _get_nc():
    if WMODE not in _CACHE:
        _CACHE[WMODE] = _build(WMODE)
    return _CACHE[WMODE]


def _prep_core_inputs(c, x, weights, biases, masks):
    b0 = c * BC
    # weights[l, b, i, j], i = ki*128 + p  ->  [t, p, ki*1024 + j]
    wc = weights[:, b0:b0 + BC].reshape(L, BC, KI, P, D)
    wc = np.ascontiguousarray(wc.transpose(0, 1, 3, 2, 4)).reshape(NT, P, KI * D)
    # last tile jb-major: [p, ki*1024 + jb*512 + j'] -> [p, jb*4096 + ki*512 + j']
    wl = wc[NT - 1].reshape(P, KI, 2, 512).transpose(0, 2, 1, 3)
    wc[NT - 1] = np.ascontiguousarray(wl).reshape(P, KI * D)
    if WMODE == "bf16":
        import ml_dtypes
        wc = wc.astype(ml_dtypes.bfloat16)
    # x[b, ki*128+p] -> [p, b*KI + ki]
    xc = x[b0:b0 + BC].reshape(BC, KI, P)
    xc = np.ascontiguousarray(xc.transpose(2, 0, 1)).reshape(P, BC * KI)
    # bias/mask [l, b, ki*128+p] -> [p, (t, {bias,mask}, ki)]
    bc = biases[:, b0:b0 + BC].reshape(L, BC, KI, P).transpose(3, 0, 1, 2)
    mc = masks[:, b0:b0 + BC].astype(np.float32).reshape(L, BC, KI, P)
    mc = mc.transpose(3, 0, 1, 2)
    # stack along a new axis after (l, b): [p, l, b, 2, ki]
    bmc = np.stack([bc, mc], axis=3)  # [p, L, BC, 2, KI]
    bmc = np.ascontiguousarray(bmc).reshape(P, NT * 2 * KI)
    return {"w": wc, "x": xc, "bm": bmc}


def _run(inputs: dict, trace: bool = False, trace_cores=None):
    x = np.asarray(inputs["x"], dtype=np.float32)
    weights = np.asarray(inputs["weights"], dtype=np.float32)
    biases = np.asarray(inputs["biases"], dtype=np.float32)
    masks = np.asarray(inputs["masks"])

    nc = _get_nc()
    in_maps = [
        _prep_core_inputs(c, x, weights, biases, masks) for c in range(NCORES)
    ]
    kw = {}
    if trace_cores is not None:
        kw["trace_cores"] = trace_cores
    res = run_bass_kernel_spmd(
        nc, in_maps, core_ids=list(range(NCORES)), trace=trace, **kw
    )
    outs = []
    for c in range(NCORES):
        oc = res.results[c]["out"]  # [P, BC*KI]
        oc = oc.reshape(P, BC, KI).transpose(1, 2, 0).reshape(BC, D)
        outs.append(oc)
    full = np.concatenate(outs, axis=0).astype(np.float32)
    return full, res


def kernel(**inputs) -> np.ndarray:
    full, _ = _run(inputs, trace=False)
    return full
